# revision 49
# baseline (speedup 1.0000x reference)
"""Trainium2 Bass kernel for nn_CausalLayer (bilinear causal mixing layer).

Math (per batch b):
    E = ae[x]                                # [L, D] gather
    S[i,j] = E_i @ w @ E_j                   # bilinear pairwise score
    coef[i,j] = (i+1)/(j+1) for i<j else 0
    res[:,j] = bx[:,j] + sum_i coef[i,j]*S[i,j]*bx[:,i]

Chunked linear-attention identity, per 128-token chunk c with
a'_i = (i+1) * (w^T e_i):
    res_j = bx_j + (1/(j+1)) * [ E_j @ M_c + sum_{i<j in c} (a'_i . e_j) bx_i ]
    M_c   = sum_{i in chunks < c} a'_i bx_i^T      ([D, H] running state)

Host prep: the fused gather table row [ae[v] | (ae@w)[v]] is gathered and
position-scaled on host, and handed to the device in the two layouts the PE
consumes: A' token-major ([ROWS, D], the Mup stationary operand) and
[Et | A't] d-major per chunk ([NCH*D, 4C], S/EM stationary operands, both
batches side by side). This removes all on-device transposes/copies for the
score path at the cost of ~1MB/core of extra DMA.

Device: the two local batches are interleaved per chunk-step. All PE
operands sit at SBUF partition base 0 (mixed-base row-tiled matmul pairs
hang TRN2); per-batch data is side-by-side on the free axis, with the M
state as one [64, 2H] PSUM accumulator (bank-aligned splits). Wire dtypes
bf16 (incl. the output, upcast on host); f32 accumulation in PSUM; the mask
is a constant 0/1 strictly-upper [128,128] tile.

Sharding: batch-parallel, 2 of 16 batches per core, no cross-core comms.
"""

import os
import sys

for _p in ("/opt/trn_rl_repo", "/root/.axon_site/_ro/trn_rl_repo"):
    if os.path.isdir(_p) and _p not in sys.path:
        sys.path.insert(0, _p)

import numpy as np

B, L, H = 16, 2048, 768
V, D = 30000, 64
NCORES = 8
BPC = B // NCORES          # batches per core
C = 128                    # chunk (tile) size along sequence
NCH = L // C               # chunks per batch
ROWS = BPC * L             # rows per core
NP2 = NCH // 2             # chunk pairs per batch

_compiled = {}

# PSUM-bank-aligned column splits for the [64, 2H] M accumulator
MUP_SPLIT = (((0, 512), (512, 768)), ((0, 256), (256, 768)))


def _build():
    key = ("v6", os.environ.get("KWARM", "9"))
    if key in _compiled:
        return _compiled[key]

    import concourse.bacc as bacc
    import concourse.bass as bass
    import concourse.mybir as mybir
    import concourse.tile as tile

    f32 = mybir.dt.float32
    bf16 = mybir.dt.bfloat16
    mult = mybir.AluOpType.mult
    add = mybir.AluOpType.add

    nc = bacc.Bacc(
        "TRN2",
        target_bir_lowering=False,
        debug=False,
        enable_asserts=False,
        num_devices=NCORES,
    )

    HD = H + D
    bx_d = nc.dram_tensor("bxap", [ROWS, HD], bf16, kind="ExternalInput").ap()
    eat_d = nc.dram_tensor("eat", [NP2 * D, 8 * C], bf16, kind="ExternalInput").ap()
    ct_d = nc.dram_tensor("consts", [C, NCH], f32, kind="ExternalInput").ap()
    mk_d = nc.dram_tensor("mask", [C, C], bf16, kind="ExternalInput").ap()
    out_d = nc.dram_tensor("out", [ROWS, H], bf16, kind="ExternalOutput").ap()

    with tile.TileContext(nc) as tc:
        with (
            tc.tile_pool(name="const", bufs=1) as cpool,
            tc.tile_pool(name="bxp", bufs=9) as bxpool,
            tc.tile_pool(name="eatp", bufs=3) as eatpool,
            tc.tile_pool(name="stp", bufs=4) as stpool,
            tc.tile_pool(name="msp", bufs=2) as mspool,
            tc.tile_pool(name="outp", bufs=4) as outpool,
            tc.tile_pool(name="ps_m", bufs=1, space="PSUM") as ps_m,
            tc.tile_pool(name="ps_out", bufs=2, space="PSUM") as ps_out,
            tc.tile_pool(name="ps_sp", bufs=1, space="PSUM") as ps_sp,
        ):
            consts_s = cpool.tile([C, NCH], f32)
            mask_s = cpool.tile([C, C], bf16)

            BX2 = {}   # (b, pair) -> [C, 2(H+D)] bf16: [bx|A'] x 2 chunks
            EAT2 = {}  # pair -> [D, 8C] bf16: two steps of [Et|A't] blocks
            ST = {}    # (b, s) -> [C, C] bf16
            SP = {}    # s -> [C, 256] f32 psum: s_p(b0), s_p(b1)
            OP = {}    # (b, s) -> [C, H] f32 psum
            OUT2 = {}  # (b, pair) -> [C, 2H] bf16
            MS = {}    # s -> [D, 2H] bf16: M(b0) | M(b1)

            def load_eat(p, eng=None):
                eng = eng if eng is not None else nc.sync
                EAT2[p] = eatpool.tile([D, 8 * C], bf16, name=f"EAT2_{p}", tag="EAT")
                eng.dma_start(out=EAT2[p][:], in_=eat_d[p * D:(p + 1) * D, :])

            def eat_view(s):
                return EAT2[s // 2][:, (s % 2) * 4 * C:(s % 2 + 1) * 4 * C]

            def load_pair(b, p, eng=None):
                eng = eng if eng is not None else nc.sync
                g = b * NCH + 2 * p
                BX2[b, p] = bxpool.tile([C, 2 * HD], bf16, name=f"BX2_{b}_{p}", tag="BX2")
                eng.dma_start(
                    out=BX2[b, p][:].rearrange("p (two h) -> p two h", two=2),
                    in_=bx_d[g * C:(g + 2) * C, :].rearrange(
                        "(two p) h -> p two h", two=2
                    ),
                )

            def ap_view(b, s):
                off = (s % 2) * HD + H
                return BX2[b, s // 2][:, off:off + D]

            def bx_view(b, s, lo=0, hi=H):
                off = (s % 2) * HD
                return BX2[b, s // 2][:, off + lo:off + hi]

            def chain_S(s):
                SP[s] = ps_sp.tile([C, 256], f32, name=f"SP_{s}", tag="SP")
                for b in (0, 1):
                    nc.tensor.matmul(
                        out=SP[s][:, b * C:(b + 1) * C],
                        lhsT=eat_view(s)[0:D, (2 * b + 1) * C:(2 * b + 2) * C],
                        rhs=eat_view(s)[0:D, 2 * b * C:(2 * b + 1) * C],
                        start=True,
                        stop=True,
                    )

            def chain_St(s):
                for b in (0, 1):
                    ST[b, s] = stpool.tile([C, C], bf16, name=f"ST_{b}_{s}", tag="ST")
                    nc.vector.tensor_tensor(
                        out=ST[b, s][:],
                        in0=SP[s][:, b * C:(b + 1) * C],
                        in1=mask_s[:],
                        op=mult,
                    )

            # prologue: only step-0/1-critical loads run ungated; everything
            # else is gated behind step-0's score tile so the shared-HBM DMA
            # rings cannot starve the critical path
            nc.sync.dma_start(out=consts_s[:], in_=ct_d[:, :])
            nc.sync.dma_start(out=mask_s[:], in_=mk_d[:, :])
            load_eat(0, nc.sync)
            load_pair(0, 0, nc.sync)
            load_pair(1, 0, nc.scalar)
            load_eat(1, nc.scalar)
            gate_t = cpool.tile([C, 1], bf16)

            # p-state warmup: the tensor engine only reaches 2.4GHz after ~3us
            # of continuous execution, so chew on scratch 512-col matmuls
            # (shared stationary operand -> no LDW gaps) while the first
            # operands stream in. The scratch PSUM tile comes from the ps_out
            # pool, whose first real tiles simply queue behind it on the PE.
            WARM = int(os.environ.get("KWARM", "9"))
            if WARM:
                wsrc = cpool.tile([C, 512], bf16)
                nc.gpsimd.memset(wsrc[:], 0.0)
                wdst = ps_out.tile([C, H], f32, name="wdst", tag="OP")
                for _ in range(WARM):
                    nc.tensor.matmul(
                        out=wdst[:, 0:512], lhsT=wsrc[:, 0:C], rhs=wsrc[:],
                        start=True, stop=True, skip_group_check=True,
                    )

            chain_S(0)
            chain_St(0)

            M_both = ps_m.tile([D, 2 * H], f32, name="M_both", tag="M_both")

            for s in range(NCH):
                nxt = s + 1
                # paced prefetch on the gpsimd ring: a tiny gpsimd read of the
                # freshest score tile delays each batch of loads until the
                # pipeline actually reaches the previous step, keeping the DMA
                # rings from racing ahead of the critical path
                if s == 0:
                    nc.gpsimd.tensor_scalar_mul(
                        out=gate_t[:], in0=ST[1, 0][:, 0:1], scalar1=1.0
                    )
                    load_pair(0, 1, nc.gpsimd)
                    load_pair(1, 1, nc.gpsimd)
                if s % 2 == 0 and s // 2 + 2 < NP2:
                    load_eat(s // 2 + 2, nc.scalar)
                pb, pp = s % 2, s // 2 + 2
                if pp < NP2:
                    load_pair(pb, pp, nc.gpsimd)

                # PE: M updates for this step (bank-aligned per-batch splits).
                # start=True arms the WHOLE 2KB psum zero-region: b1's (0,256)
                # shares a bank with b0's (512,768), so it must NOT re-arm it
                # (its bytes are already pending from b0's start, making its
                # first write an overwrite as required).
                if s < NCH - 1:
                    MS[nxt] = mspool.tile([D, 2 * H], bf16, name=f"MS_{nxt}", tag="MS")
                    for b in (0, 1):
                        for lo, hi in MUP_SPLIT[b]:
                            nc.tensor.matmul(
                                out=M_both[:, b * H + lo:b * H + hi],
                                lhsT=ap_view(b, s),
                                rhs=bx_view(b, s, lo, hi),
                                start=(s == 0 and not (b == 1 and lo == 0)),
                                stop=True,
                                skip_group_check=True,
                            )
                    nc.scalar.copy(out=MS[nxt][:], in_=M_both[:])

                # PE: score matmuls for next step
                if nxt < NCH:
                    chain_S(nxt)
                    chain_St(nxt)

                # PE: output accumulation + final AXPY per batch
                for b in (0, 1):
                    OP[b, s] = ps_out.tile([C, H], f32, name=f"OP_{b}_{s}", tag="OP")
                    if s > 0:
                        for lo, hi in ((0, 512), (512, H)):
                            nc.tensor.matmul(
                                out=OP[b, s][:, lo:hi],
                                lhsT=eat_view(s)[0:D, 2 * b * C:(2 * b + 1) * C],
                                rhs=MS[s][0:D, b * H + lo:b * H + hi],
                                start=True,
                                stop=False,
                            )
                    for lo, hi in ((0, 512), (512, H)):
                        nc.tensor.matmul(
                            out=OP[b, s][:, lo:hi],
                            lhsT=ST[b, s][:],
                            rhs=bx_view(b, s, lo, hi),
                            start=(s == 0),
                            stop=True,
                        )
                    # res = OP * (1/(j+1)) + bx -> bf16 (DVE)
                    if s % 2 == 0:
                        OUT2[b, s // 2] = outpool.tile(
                            [C, 2 * H], bf16, name=f"OUT2_{b}_{s // 2}", tag="OUT2"
                        )
                    ov = OUT2[b, s // 2][:, (s % 2) * H:(s % 2 + 1) * H]
                    nc.vector.scalar_tensor_tensor(
                        out=ov,
                        in0=OP[b, s][:],
                        scalar=consts_s[:, s:s + 1],
                        in1=bx_view(b, s),
                        op0=mult,
                        op1=add,
                    )

                # out DMA per completed pair
                if s % 2 == 1:
                    for b in (0, 1):
                        g = b * NCH + s
                        nc.sync.dma_start(
                            out=out_d[(g - 1) * C:(g + 1) * C, :].rearrange(
                                "(two p) h -> p two h", two=2
                            ),
                            in_=OUT2[b, s // 2][:].rearrange(
                                "p (two h) -> p two h", two=2
                            ),
                        )

    # Adjacent PE matmuls sharing a stationary operand reload it redundantly;
    # mark the second of each such pair as pre-loaded.
    for blk in nc.m.functions[0].blocks:
        last = None
        for inst in blk.instructions:
            if getattr(inst, "engine", None) != mybir.EngineType.PE:
                continue
            if not isinstance(inst, mybir.InstMatmult):
                if isinstance(inst, (mybir.InstLdweights,)):
                    last = None
                continue
            if (
                last is not None
                and not inst.is_transpose
                and not last.is_transpose
                and inst.ins[1].memref == last.ins[1].memref
                and inst.ins[1].offset == last.ins[1].offset
                and inst.ins[1].ap == last.ins[1].ap
            ):
                inst.ldweights = True
            last = inst

    nc.compile()
    _compiled[key] = nc
    return nc


def _np_consts():
    j = np.arange(L, dtype=np.float64)
    inv = (1.0 / (j + 1.0)).astype(np.float32).reshape(NCH, C).T
    consts = np.ascontiguousarray(inv)  # [C, NCH], col c = 1/(c*128+i+1)
    mask01 = np.triu(np.ones((C, C), np.float32), 1)
    return consts, mask01


def _in_maps(bert_x, x, ae, w):
    import ml_dtypes

    bert_x = np.asarray(bert_x, dtype=np.float32)
    x = np.asarray(x)
    ae = np.asarray(ae, dtype=np.float32)
    w = np.asarray(w, dtype=np.float32)

    eaw = np.concatenate([ae, ae @ w], axis=1)          # [V, 2D] f32
    EA = eaw[x]                                         # [B, L, 2D] f32
    scale_i = (np.arange(L, dtype=np.float64) + 1.0).astype(np.float32)
    EA[:, :, D:] *= scale_i[None, :, None]
    EAb = EA.astype(ml_dtypes.bfloat16)                 # [B, L, 2D]
    bxb = bert_x.astype(ml_dtypes.bfloat16)

    # merged [bx | A'] rows so each chunk pair is one DMA
    bxap = np.ascontiguousarray(
        np.concatenate([bxb, EAb[:, :, D:]], axis=2)    # [B, L, H+D]
    )

    # d-major per-chunk stationary blocks, same bf16 values as EAb, packed
    # two steps per row-block: eat[core, p*D:(p+1)*D, :] =
    # [Et(b0)|A't(b0)|Et(b1)|A't(b1)] of step 2p, then of step 2p+1
    EAc = EAb.reshape(NCORES, BPC, NCH, C, 2 * D)
    blocks = np.transpose(EAc, (0, 2, 1, 4, 3))         # [cores,NCH,BPC,2D,C]
    blocks = blocks.reshape(NCORES, NCH, BPC * 2, D, C)
    eat = np.transpose(blocks, (0, 1, 3, 2, 4)).reshape(NCORES, NCH, D, 4 * C)
    eat = eat.reshape(NCORES, NP2, 2, D, 4 * C)
    eat = np.transpose(eat, (0, 1, 3, 2, 4)).reshape(NCORES, NP2 * D, 8 * C)
    eat = np.ascontiguousarray(eat)

    consts, mask01 = _np_consts()
    mask_b = np.ascontiguousarray(mask01.astype(ml_dtypes.bfloat16))

    maps = []
    for k in range(NCORES):
        maps.append(
            {
                "bxap": bxap[k * BPC:(k + 1) * BPC].reshape(ROWS, H + D),
                "eat": eat[k],
                "consts": consts,
                "mask": mask_b,
            }
        )
    return maps


def _run(bert_x, x, ae, w, trace=False):
    from concourse import bass_utils

    nc = _build()
    maps = _in_maps(bert_x, x, ae, w)
    res = bass_utils.run_bass_kernel_spmd(
        nc, maps, core_ids=list(range(NCORES)), trace=trace
    )
    out = np.concatenate(
        [
            res.results[k]["out"].astype(np.float32).reshape(BPC, L, H)
            for k in range(NCORES)
        ],
        axis=0,
    )
    return out, res


def kernel(bert_x, x, ae, w):
    out, _ = _run(bert_x, x, ae, w, trace=False)
    return out


# revision 50
# speedup vs baseline: 1.0760x; 1.0760x over previous
"""Trainium2 Bass kernel for nn_CausalLayer (bilinear causal mixing layer).

Math (per batch b):
    E = ae[x]                                # [L, D] gather
    S[i,j] = E_i @ w @ E_j                   # bilinear pairwise score
    coef[i,j] = (i+1)/(j+1) for i<j else 0
    res[:,j] = bx[:,j] + sum_i coef[i,j]*S[i,j]*bx[:,i]

Chunked linear-attention identity, per 128-token chunk c with
a'_i = (i+1) * (w^T e_i):
    res_j = bx_j + (1/(j+1)) * [ E_j @ M_c + sum_{i<j in c} (a'_i . e_j) bx_i ]
    M_c   = sum_{i in chunks < c} a'_i bx_i^T      ([D, H] running state)

Host prep: EAs[b,i] = [ae[x[b,i]] | (ae@w)[x[b,i]] * (i+1)] in bf16 (table
fused + gathered + position-scaled on host; device DMA bytes are unchanged
versus an on-device indirect gather, but the ~1us/chunk software-descriptor
overhead disappears).

Device: the two local batches are interleaved per chunk-step. All PE
operands sit at SBUF partition base 0 (mixed-base row-tiled matmul pairs
hang TRN2). Per-batch data is laid out side-by-side on the free axis:
EAT[s] = [Et(b0) | A't(b0) | Et(b1) | A't(b1)] as a [64, 512] tile built by
four [64,128] PE transposes, M state as one [64, 2H] PSUM accumulator with
bank-aligned splits. Wire dtypes bf16 (incl. the output, upcast on host);
f32 accumulation in PSUM; the mask is a constant 0/1 strictly-upper
[128,128] tile. The PE stream is software-pipelined one step ahead so the
tensor engine stays continuously busy and reaches its max p-state.

Sharding: batch-parallel, 2 of 16 batches per core, no cross-core comms.
"""

import os
import sys

for _p in ("/opt/trn_rl_repo", "/root/.axon_site/_ro/trn_rl_repo"):
    if os.path.isdir(_p) and _p not in sys.path:
        sys.path.insert(0, _p)

import numpy as np

B, L, H = 16, 2048, 768
V, D = 30000, 64
NCORES = 8
BPC = B // NCORES          # batches per core
C = 128                    # chunk (tile) size along sequence
NCH = L // C               # chunks per batch
ROWS = BPC * L             # rows per core
NP2 = NCH // 2             # chunk pairs per batch

_compiled = {}

# PSUM-bank-aligned column splits for the [64, 2H] M accumulator
MUP_SPLIT = (((0, 512), (512, 768)), ((0, 256), (256, 768)))


def _build():
    key = "v3f"
    if key in _compiled:
        return _compiled[key]

    import concourse.bacc as bacc
    import concourse.bass as bass
    import concourse.mybir as mybir
    import concourse.tile as tile
    from concourse.masks import make_identity

    f32 = mybir.dt.float32
    bf16 = mybir.dt.bfloat16
    mult = mybir.AluOpType.mult
    add = mybir.AluOpType.add

    nc = bacc.Bacc(
        "TRN2",
        target_bir_lowering=False,
        debug=False,
        enable_asserts=False,
        num_devices=NCORES,
    )

    bx_d = nc.dram_tensor("bx", [ROWS, H], bf16, kind="ExternalInput").ap()
    eas_d = nc.dram_tensor("eas", [ROWS, 2 * D], bf16, kind="ExternalInput").ap()
    ct_d = nc.dram_tensor("consts", [C, NCH], f32, kind="ExternalInput").ap()
    mk_d = nc.dram_tensor("mask", [C, C], bf16, kind="ExternalInput").ap()
    out_d = nc.dram_tensor("out", [ROWS, H], bf16, kind="ExternalOutput").ap()

    with tile.TileContext(nc) as tc:
        with (
            tc.tile_pool(name="const", bufs=1) as cpool,
            tc.tile_pool(name="bxp", bufs=6) as bxpool,
            tc.tile_pool(name="easp", bufs=6) as easpool,
            tc.tile_pool(name="eatp", bufs=4) as eatpool,
            tc.tile_pool(name="stp", bufs=4) as stpool,
            tc.tile_pool(name="msp", bufs=2) as mspool,
            tc.tile_pool(name="outp", bufs=4) as outpool,
            tc.tile_pool(name="ps_m", bufs=1, space="PSUM") as ps_m,
            tc.tile_pool(name="ps_out", bufs=2, space="PSUM") as ps_out,
            tc.tile_pool(name="ps_tps", bufs=1, space="PSUM") as ps_tps,
        ):
            ident16 = cpool.tile([C, C], bf16)
            make_identity(nc, ident16[:])
            consts_s = cpool.tile([C, NCH], f32)
            mask_s = cpool.tile([C, C], bf16)

            BX2 = {}   # (b, pair) -> [C, 2H] bf16
            EAS2 = {}  # (b, pair) -> [C, 4D] bf16
            EAT = {}   # s -> [D, 4C] bf16: [Et(b0)|A't(b0)|Et(b1)|A't(b1)]
            ST = {}    # (b, s) -> [C, C] bf16
            TPS = {}   # s -> [C, 512] f32 psum: s_p(b0), s_p(b1), T-blocks
            OP = {}    # (b, s) -> [C, H] f32 psum
            OUT2 = {}  # (b, pair) -> [C, 2H] bf16
            MS = {}    # s -> [D, 2H] bf16: M(b0) | M(b1)

            def load_pair(b, p, eng=None):
                eng = eng if eng is not None else nc.sync
                g = b * NCH + 2 * p
                EAS2[b, p] = easpool.tile(
                    [C, 4 * D], bf16, name=f"EAS2_{b}_{p}", tag="EAS2"
                )
                eng.dma_start(
                    out=EAS2[b, p][:].rearrange("p (two d) -> p two d", two=2),
                    in_=eas_d[g * C:(g + 2) * C, :].rearrange(
                        "(two p) d -> p two d", two=2
                    ),
                )
                BX2[b, p] = bxpool.tile([C, 2 * H], bf16, name=f"BX2_{b}_{p}", tag="BX2")
                eng.dma_start(
                    out=BX2[b, p][:].rearrange("p (two h) -> p two h", two=2),
                    in_=bx_d[g * C:(g + 2) * C, :].rearrange(
                        "(two p) h -> p two h", two=2
                    ),
                )

            def eas_view(b, s):
                off = (s % 2) * 2 * D
                return EAS2[b, s // 2][:, off:off + 2 * D]

            def bx_view(b, s, lo=0, hi=H):
                off = (s % 2) * H
                return BX2[b, s // 2][:, off + lo:off + hi]

            def chain_T(s):
                """Four [64,128] transposes into the step's TPS tile, all at
                partition base 0: bf16 view cols = b*2C + half*C."""
                TPS[s] = ps_tps.tile([C, 512], f32, name=f"TPS_{s}", tag="TPS")
                tv = TPS[s][:, 256:512].bitcast(bf16)  # [C, 512] bf16 view
                for b in (0, 1):
                    for half in (0, 1):  # 0 = E, 1 = A'
                        nc.tensor.transpose(
                            out=tv[0:D, (2 * b + half) * C:(2 * b + half + 1) * C],
                            in_=eas_view(b, s)[:, half * D:(half + 1) * D],
                            identity=ident16[:],
                        )

            def chain_EAt(s):
                tv = TPS[s][:, 256:512].bitcast(bf16)
                EAT[s] = eatpool.tile([D, 4 * C], bf16, name=f"EAT_{s}", tag="EAT")
                nc.scalar.copy(out=EAT[s][:], in_=tv[0:D, 0:4 * C])

            def chain_S(s):
                for b in (0, 1):
                    nc.tensor.matmul(
                        out=TPS[s][:, b * C:(b + 1) * C],
                        lhsT=EAT[s][0:D, (2 * b + 1) * C:(2 * b + 2) * C],
                        rhs=EAT[s][0:D, 2 * b * C:(2 * b + 1) * C],
                        start=True,
                        stop=True,
                    )

            def chain_St(s):
                for b in (0, 1):
                    ST[b, s] = stpool.tile([C, C], bf16, name=f"ST_{b}_{s}", tag="ST")
                    nc.vector.tensor_tensor(
                        out=ST[b, s][:],
                        in0=TPS[s][:, b * C:(b + 1) * C],
                        in1=mask_s[:],
                        op=mult,
                    )

            # prologue: spread the initial loads over two DMA queues so the
            # critical EAS2(b0,0) lands first
            g00 = 0
            EAS2[0, 0] = easpool.tile([C, 4 * D], bf16, name="EAS2_0_0", tag="EAS2")
            nc.sync.dma_start(
                out=EAS2[0, 0][:].rearrange("p (two d) -> p two d", two=2),
                in_=eas_d[g00 * C:(g00 + 2) * C, :].rearrange(
                    "(two p) d -> p two d", two=2
                ),
            )
            nc.sync.dma_start(out=consts_s[:], in_=ct_d[:, :])
            nc.sync.dma_start(out=mask_s[:], in_=mk_d[:, :])
            BX2[0, 0] = bxpool.tile([C, 2 * H], bf16, name="BX2_0_0", tag="BX2")
            nc.sync.dma_start(
                out=BX2[0, 0][:].rearrange("p (two h) -> p two h", two=2),
                in_=bx_d[g00 * C:(g00 + 2) * C, :].rearrange(
                    "(two p) h -> p two h", two=2
                ),
            )
            load_pair(1, 0, nc.scalar)
            load_pair(0, 1, nc.sync)
            load_pair(1, 1, nc.scalar)
            chain_T(0)
            chain_EAt(0)
            chain_S(0)
            chain_St(0)

            M_both = ps_m.tile([D, 2 * H], f32, name="M_both", tag="M_both")

            for s in range(NCH):
                nxt = s + 1
                # prefetch 2 pairs ahead
                if s % 2 == 0:
                    p = s // 2 + 2
                    if p < NP2:
                        for b in (0, 1):
                            load_pair(b, p)

                # PE: transposes for next step (+ ACT copy)
                if nxt < NCH:
                    chain_T(nxt)
                    chain_EAt(nxt)

                # PE: M updates for this step (bank-aligned per-batch splits).
                # start=True arms the WHOLE 2KB psum zero-region: b1's (0,256)
                # shares a bank with b0's (512,768), so it must NOT re-arm it
                # (its bytes are already pending from b0's start, making its
                # first write an overwrite as required).
                if s < NCH - 1:
                    MS[nxt] = mspool.tile([D, 2 * H], bf16, name=f"MS_{nxt}", tag="MS")
                    for b in (0, 1):
                        for lo, hi in MUP_SPLIT[b]:
                            nc.tensor.matmul(
                                out=M_both[:, b * H + lo:b * H + hi],
                                lhsT=eas_view(b, s)[:, D:2 * D],
                                rhs=bx_view(b, s, lo, hi),
                                start=(s == 0 and not (b == 1 and lo == 0)),
                                stop=True,
                                skip_group_check=True,
                            )
                    nc.scalar.copy(out=MS[nxt][:], in_=M_both[:])

                # PE: score matmuls for next step
                if nxt < NCH:
                    chain_S(nxt)
                    chain_St(nxt)

                # PE: output accumulation + final AXPY per batch
                for b in (0, 1):
                    OP[b, s] = ps_out.tile([C, H], f32, name=f"OP_{b}_{s}", tag="OP")
                    if s > 0:
                        for lo, hi in ((0, 512), (512, H)):
                            nc.tensor.matmul(
                                out=OP[b, s][:, lo:hi],
                                lhsT=EAT[s][0:D, 2 * b * C:(2 * b + 1) * C],
                                rhs=MS[s][0:D, b * H + lo:b * H + hi],
                                start=True,
                                stop=False,
                            )
                    for lo, hi in ((0, 512), (512, H)):
                        nc.tensor.matmul(
                            out=OP[b, s][:, lo:hi],
                            lhsT=ST[b, s][:],
                            rhs=bx_view(b, s, lo, hi),
                            start=(s == 0),
                            stop=True,
                        )
                    # res = OP * (1/(j+1)) + bx -> bf16 (DVE)
                    if s % 2 == 0:
                        OUT2[b, s // 2] = outpool.tile(
                            [C, 2 * H], bf16, name=f"OUT2_{b}_{s // 2}", tag="OUT2"
                        )
                    ov = OUT2[b, s // 2][:, (s % 2) * H:(s % 2 + 1) * H]
                    nc.vector.scalar_tensor_tensor(
                        out=ov,
                        in0=OP[b, s][:],
                        scalar=consts_s[:, s:s + 1],
                        in1=bx_view(b, s),
                        op0=mult,
                        op1=add,
                    )

                # out DMA per completed pair
                if s % 2 == 1:
                    for b in (0, 1):
                        g = b * NCH + s
                        nc.sync.dma_start(
                            out=out_d[(g - 1) * C:(g + 1) * C, :].rearrange(
                                "(two p) h -> p two h", two=2
                            ),
                            in_=OUT2[b, s // 2][:].rearrange(
                                "p (two h) -> p two h", two=2
                            ),
                        )

    # Adjacent PE matmuls sharing a stationary operand reload it redundantly;
    # mark the second of each such pair as pre-loaded.
    for blk in nc.m.functions[0].blocks:
        last = None
        for inst in blk.instructions:
            if getattr(inst, "engine", None) != mybir.EngineType.PE:
                continue
            if not isinstance(inst, mybir.InstMatmult):
                if isinstance(inst, (mybir.InstLdweights,)):
                    last = None
                continue
            if (
                last is not None
                and not inst.is_transpose
                and not last.is_transpose
                and inst.ins[1].memref == last.ins[1].memref
                and inst.ins[1].offset == last.ins[1].offset
                and inst.ins[1].ap == last.ins[1].ap
            ):
                inst.ldweights = True
            last = inst

    nc.compile()
    _compiled[key] = nc
    return nc


def _np_consts():
    j = np.arange(L, dtype=np.float64)
    inv = (1.0 / (j + 1.0)).astype(np.float32).reshape(NCH, C).T
    consts = np.ascontiguousarray(inv)  # [C, NCH], col c = 1/(c*128+i+1)
    mask01 = np.triu(np.ones((C, C), np.float32), 1)
    return consts, mask01


def _in_maps(bert_x, x, ae, w):
    import ml_dtypes

    bert_x = np.asarray(bert_x, dtype=np.float32)
    x = np.asarray(x)
    ae = np.asarray(ae, dtype=np.float32)
    w = np.asarray(w, dtype=np.float32)

    eaw = np.concatenate([ae, ae @ w], axis=1)          # [V, 2D] f32
    EA = eaw[x]                                         # [B, L, 2D] f32
    scale_i = (np.arange(L, dtype=np.float64) + 1.0).astype(np.float32)
    EA[:, :, D:] *= scale_i[None, :, None]
    EAs = np.ascontiguousarray(EA.astype(ml_dtypes.bfloat16))
    bxb = np.ascontiguousarray(bert_x.astype(ml_dtypes.bfloat16))

    consts, mask01 = _np_consts()
    mask_b = np.ascontiguousarray(mask01.astype(ml_dtypes.bfloat16))

    maps = []
    for k in range(NCORES):
        maps.append(
            {
                "bx": bxb[k * BPC:(k + 1) * BPC].reshape(ROWS, H),
                "eas": EAs[k * BPC:(k + 1) * BPC].reshape(ROWS, 2 * D),
                "consts": consts,
                "mask": mask_b,
            }
        )
    return maps


def _run(bert_x, x, ae, w, trace=False):
    from concourse import bass_utils

    nc = _build()
    maps = _in_maps(bert_x, x, ae, w)
    res = bass_utils.run_bass_kernel_spmd(
        nc, maps, core_ids=list(range(NCORES)), trace=trace
    )
    out = np.concatenate(
        [
            res.results[k]["out"].astype(np.float32).reshape(BPC, L, H)
            for k in range(NCORES)
        ],
        axis=0,
    )
    return out, res


def kernel(bert_x, x, ae, w):
    out, _ = _run(bert_x, x, ae, w, trace=False)
    return out


# revision 51
# speedup vs baseline: 1.0772x; 1.0011x over previous
"""Trainium2 Bass kernel for nn_CausalLayer (bilinear causal mixing layer).

Math (per batch b):
    E = ae[x]                                # [L, D] gather
    S[i,j] = E_i @ w @ E_j                   # bilinear pairwise score
    coef[i,j] = (i+1)/(j+1) for i<j else 0
    res[:,j] = bx[:,j] + sum_i coef[i,j]*S[i,j]*bx[:,i]

Chunked linear-attention identity, per 128-token chunk c with
a'_i = (i+1) * (w^T e_i):
    res_j = bx_j + (1/(j+1)) * [ E_j @ M_c + sum_{i<j in c} (a'_i . e_j) bx_i ]
    M_c   = sum_{i in chunks < c} a'_i bx_i^T      ([D, H] running state)

Host prep: EAs[b,i] = [ae[x[b,i]] | (ae@w)[x[b,i]] * (i+1)] in bf16 (table
fused + gathered + position-scaled on host; device DMA bytes are unchanged
versus an on-device indirect gather, but the ~1us/chunk software-descriptor
overhead disappears).

Device: the two local batches are interleaved per chunk-step. All PE
operands sit at SBUF partition base 0 (mixed-base row-tiled matmul pairs
hang TRN2). Per-batch data is laid out side-by-side on the free axis:
EAT[s] = [Et(b0) | A't(b0) | Et(b1) | A't(b1)] as a [64, 512] tile built by
four [64,128] PE transposes, M state as one [64, 2H] PSUM accumulator with
bank-aligned splits. Wire dtypes bf16 (incl. the output, upcast on host);
f32 accumulation in PSUM; the mask is a constant 0/1 strictly-upper
[128,128] tile. The PE stream is software-pipelined one step ahead so the
tensor engine stays continuously busy and reaches its max p-state.

Sharding: batch-parallel, 2 of 16 batches per core, no cross-core comms.
"""

import os
import sys

for _p in ("/opt/trn_rl_repo", "/root/.axon_site/_ro/trn_rl_repo"):
    if os.path.isdir(_p) and _p not in sys.path:
        sys.path.insert(0, _p)

import numpy as np

B, L, H = 16, 2048, 768
V, D = 30000, 64
NCORES = 8
BPC = B // NCORES          # batches per core
C = 128                    # chunk (tile) size along sequence
NCH = L // C               # chunks per batch
ROWS = BPC * L             # rows per core
NP2 = NCH // 2             # chunk pairs per batch

_compiled = {}

# PSUM-bank-aligned column splits for the [64, 2H] M accumulator
MUP_SPLIT = (((0, 512), (512, 768)), ((0, 256), (256, 768)))


def _build():
    key = "v3f"
    if key in _compiled:
        return _compiled[key]

    import concourse.bacc as bacc
    import concourse.bass as bass
    import concourse.mybir as mybir
    import concourse.tile as tile
    from concourse.masks import make_identity

    f32 = mybir.dt.float32
    bf16 = mybir.dt.bfloat16
    mult = mybir.AluOpType.mult
    add = mybir.AluOpType.add

    nc = bacc.Bacc(
        "TRN2",
        target_bir_lowering=False,
        debug=False,
        enable_asserts=False,
        num_devices=NCORES,
    )

    bx_d = nc.dram_tensor("bx", [ROWS, H], bf16, kind="ExternalInput").ap()
    eas_d = nc.dram_tensor("eas", [ROWS, 2 * D], bf16, kind="ExternalInput").ap()
    ct_d = nc.dram_tensor("consts", [C, NCH], f32, kind="ExternalInput").ap()
    mk_d = nc.dram_tensor("mask", [C, C], bf16, kind="ExternalInput").ap()
    out_d = nc.dram_tensor("out", [ROWS, H], bf16, kind="ExternalOutput").ap()

    with tile.TileContext(nc) as tc:
        with (
            tc.tile_pool(name="const", bufs=1) as cpool,
            tc.tile_pool(name="bxp", bufs=6) as bxpool,
            tc.tile_pool(name="easp", bufs=6) as easpool,
            tc.tile_pool(name="eatp", bufs=4) as eatpool,
            tc.tile_pool(name="stp", bufs=4) as stpool,
            tc.tile_pool(name="msp", bufs=2) as mspool,
            tc.tile_pool(name="outp", bufs=4) as outpool,
            tc.tile_pool(name="ps_m", bufs=1, space="PSUM") as ps_m,
            tc.tile_pool(name="ps_out", bufs=2, space="PSUM") as ps_out,
            tc.tile_pool(name="ps_tps", bufs=1, space="PSUM") as ps_tps,
        ):
            ident16 = cpool.tile([C, C], bf16)
            make_identity(nc, ident16[:])
            consts_s = cpool.tile([C, NCH], f32)
            mask_s = cpool.tile([C, C], bf16)

            BX2 = {}   # (b, pair) -> [C, 2H] bf16
            EAS2 = {}  # (b, pair) -> [C, 4D] bf16
            EAT = {}   # s -> [D, 4C] bf16: [Et(b0)|A't(b0)|Et(b1)|A't(b1)]
            ST = {}    # (b, s) -> [C, C] bf16
            TPS = {}   # s -> [C, 512] f32 psum: s_p(b0), s_p(b1), T-blocks
            OP = {}    # (b, s) -> [C, H] f32 psum
            OUT2 = {}  # (b, pair) -> [C, 2H] bf16
            MS = {}    # s -> [D, 2H] bf16: M(b0) | M(b1)

            def load_pair(b, p, eng=None):
                eng = eng if eng is not None else nc.sync
                g = b * NCH + 2 * p
                EAS2[b, p] = easpool.tile(
                    [C, 4 * D], bf16, name=f"EAS2_{b}_{p}", tag="EAS2"
                )
                eng.dma_start(
                    out=EAS2[b, p][:].rearrange("p (two d) -> p two d", two=2),
                    in_=eas_d[g * C:(g + 2) * C, :].rearrange(
                        "(two p) d -> p two d", two=2
                    ),
                )
                BX2[b, p] = bxpool.tile([C, 2 * H], bf16, name=f"BX2_{b}_{p}", tag="BX2")
                eng.dma_start(
                    out=BX2[b, p][:].rearrange("p (two h) -> p two h", two=2),
                    in_=bx_d[g * C:(g + 2) * C, :].rearrange(
                        "(two p) h -> p two h", two=2
                    ),
                )

            def eas_view(b, s):
                off = (s % 2) * 2 * D
                return EAS2[b, s // 2][:, off:off + 2 * D]

            def bx_view(b, s, lo=0, hi=H):
                off = (s % 2) * H
                return BX2[b, s // 2][:, off + lo:off + hi]

            def chain_T(s):
                """Four [64,128] transposes into the step's TPS tile, all at
                partition base 0: bf16 view cols = b*2C + half*C."""
                TPS[s] = ps_tps.tile([C, 512], f32, name=f"TPS_{s}", tag="TPS")
                tv = TPS[s][:, 256:512].bitcast(bf16)  # [C, 512] bf16 view
                for b in (0, 1):
                    for half in (0, 1):  # 0 = E, 1 = A'
                        nc.tensor.transpose(
                            out=tv[0:D, (2 * b + half) * C:(2 * b + half + 1) * C],
                            in_=eas_view(b, s)[:, half * D:(half + 1) * D],
                            identity=ident16[:],
                        )

            def chain_EAt(s):
                tv = TPS[s][:, 256:512].bitcast(bf16)
                EAT[s] = eatpool.tile([D, 4 * C], bf16, name=f"EAT_{s}", tag="EAT")
                nc.scalar.copy(out=EAT[s][:], in_=tv[0:D, 0:4 * C])

            def chain_S(s):
                for b in (0, 1):
                    nc.tensor.matmul(
                        out=TPS[s][:, b * C:(b + 1) * C],
                        lhsT=EAT[s][0:D, (2 * b + 1) * C:(2 * b + 2) * C],
                        rhs=EAT[s][0:D, 2 * b * C:(2 * b + 1) * C],
                        start=True,
                        stop=True,
                    )

            def chain_St(s):
                for b in (0, 1):
                    ST[b, s] = stpool.tile([C, C], bf16, name=f"ST_{b}_{s}", tag="ST")
                    nc.vector.tensor_tensor(
                        out=ST[b, s][:],
                        in0=TPS[s][:, b * C:(b + 1) * C],
                        in1=mask_s[:],
                        op=mult,
                    )

            # prologue: spread the initial loads over two DMA queues so the
            # critical EAS2(b0,0) lands first
            g00 = 0
            EAS2[0, 0] = easpool.tile([C, 4 * D], bf16, name="EAS2_0_0", tag="EAS2")
            nc.sync.dma_start(
                out=EAS2[0, 0][:].rearrange("p (two d) -> p two d", two=2),
                in_=eas_d[g00 * C:(g00 + 2) * C, :].rearrange(
                    "(two p) d -> p two d", two=2
                ),
            )
            nc.sync.dma_start(out=consts_s[:], in_=ct_d[:, :])
            nc.sync.dma_start(out=mask_s[:], in_=mk_d[:, :])
            BX2[0, 0] = bxpool.tile([C, 2 * H], bf16, name="BX2_0_0", tag="BX2")
            nc.sync.dma_start(
                out=BX2[0, 0][:].rearrange("p (two h) -> p two h", two=2),
                in_=bx_d[g00 * C:(g00 + 2) * C, :].rearrange(
                    "(two p) h -> p two h", two=2
                ),
            )
            load_pair(1, 0, nc.scalar)
            load_pair(0, 1, nc.sync)
            load_pair(1, 1, nc.scalar)
            chain_T(0)
            chain_EAt(0)
            chain_S(0)
            chain_St(0)

            M_both = ps_m.tile([D, 2 * H], f32, name="M_both", tag="M_both")

            for s in range(NCH):
                nxt = s + 1
                # prefetch 2 pairs ahead
                if s % 2 == 0:
                    p = s // 2 + 2
                    if p < NP2:
                        for b in (0, 1):
                            load_pair(b, p)

                # PE: next step's transposes interleaved with this step's
                # M updates so each tiny transpose's weight-load hides behind
                # a long Mup stream. Bank-arming order of the M splits is
                # preserved (b0's (512,768) start precedes b1's (0,256)).
                have_t = nxt < NCH
                have_m = s < NCH - 1
                if have_t:
                    TPS[nxt] = ps_tps.tile([C, 512], f32, name=f"TPS_{nxt}", tag="TPS")
                    tv_n = TPS[nxt][:, 256:512].bitcast(bf16)
                if have_m:
                    MS[nxt] = mspool.tile([D, 2 * H], bf16, name=f"MS_{nxt}", tag="MS")
                for b in (0, 1):
                    for half in (0, 1):
                        if have_t:
                            nc.tensor.transpose(
                                out=tv_n[0:D, (2 * b + half) * C:(2 * b + half + 1) * C],
                                in_=eas_view(b, nxt)[:, half * D:(half + 1) * D],
                                identity=ident16[:],
                            )
                        if have_m:
                            lo, hi = MUP_SPLIT[b][half]
                            nc.tensor.matmul(
                                out=M_both[:, b * H + lo:b * H + hi],
                                lhsT=eas_view(b, s)[:, D:2 * D],
                                rhs=bx_view(b, s, lo, hi),
                                start=(s == 0 and not (b == 1 and lo == 0)),
                                stop=True,
                                skip_group_check=True,
                            )
                if have_t:
                    chain_EAt(nxt)
                if have_m:
                    nc.scalar.copy(out=MS[nxt][:], in_=M_both[:])

                # PE: score matmuls for next step
                if nxt < NCH:
                    chain_S(nxt)
                    chain_St(nxt)

                # PE: output accumulation + final AXPY per batch
                for b in (0, 1):
                    OP[b, s] = ps_out.tile([C, H], f32, name=f"OP_{b}_{s}", tag="OP")
                    if s > 0:
                        for lo, hi in ((0, 512), (512, H)):
                            nc.tensor.matmul(
                                out=OP[b, s][:, lo:hi],
                                lhsT=EAT[s][0:D, 2 * b * C:(2 * b + 1) * C],
                                rhs=MS[s][0:D, b * H + lo:b * H + hi],
                                start=True,
                                stop=False,
                            )
                    for lo, hi in ((0, 512), (512, H)):
                        nc.tensor.matmul(
                            out=OP[b, s][:, lo:hi],
                            lhsT=ST[b, s][:],
                            rhs=bx_view(b, s, lo, hi),
                            start=(s == 0),
                            stop=True,
                        )
                    # res = OP * (1/(j+1)) + bx -> bf16 (DVE)
                    if s % 2 == 0:
                        OUT2[b, s // 2] = outpool.tile(
                            [C, 2 * H], bf16, name=f"OUT2_{b}_{s // 2}", tag="OUT2"
                        )
                    ov = OUT2[b, s // 2][:, (s % 2) * H:(s % 2 + 1) * H]
                    nc.vector.scalar_tensor_tensor(
                        out=ov,
                        in0=OP[b, s][:],
                        scalar=consts_s[:, s:s + 1],
                        in1=bx_view(b, s),
                        op0=mult,
                        op1=add,
                    )

                # out DMA per completed pair
                if s % 2 == 1:
                    for b in (0, 1):
                        g = b * NCH + s
                        nc.sync.dma_start(
                            out=out_d[(g - 1) * C:(g + 1) * C, :].rearrange(
                                "(two p) h -> p two h", two=2
                            ),
                            in_=OUT2[b, s // 2][:].rearrange(
                                "p (two h) -> p two h", two=2
                            ),
                        )

    # Adjacent PE matmuls sharing a stationary operand reload it redundantly;
    # mark the second of each such pair as pre-loaded.
    for blk in nc.m.functions[0].blocks:
        last = None
        for inst in blk.instructions:
            if getattr(inst, "engine", None) != mybir.EngineType.PE:
                continue
            if not isinstance(inst, mybir.InstMatmult):
                if isinstance(inst, (mybir.InstLdweights,)):
                    last = None
                continue
            if (
                last is not None
                and not inst.is_transpose
                and not last.is_transpose
                and inst.ins[1].memref == last.ins[1].memref
                and inst.ins[1].offset == last.ins[1].offset
                and inst.ins[1].ap == last.ins[1].ap
            ):
                inst.ldweights = True
            last = inst

    nc.compile()
    _compiled[key] = nc
    return nc


def _np_consts():
    j = np.arange(L, dtype=np.float64)
    inv = (1.0 / (j + 1.0)).astype(np.float32).reshape(NCH, C).T
    consts = np.ascontiguousarray(inv)  # [C, NCH], col c = 1/(c*128+i+1)
    mask01 = np.triu(np.ones((C, C), np.float32), 1)
    return consts, mask01


def _in_maps(bert_x, x, ae, w):
    import ml_dtypes

    bert_x = np.asarray(bert_x, dtype=np.float32)
    x = np.asarray(x)
    ae = np.asarray(ae, dtype=np.float32)
    w = np.asarray(w, dtype=np.float32)

    eaw = np.concatenate([ae, ae @ w], axis=1)          # [V, 2D] f32
    EA = eaw[x]                                         # [B, L, 2D] f32
    scale_i = (np.arange(L, dtype=np.float64) + 1.0).astype(np.float32)
    EA[:, :, D:] *= scale_i[None, :, None]
    EAs = np.ascontiguousarray(EA.astype(ml_dtypes.bfloat16))
    bxb = np.ascontiguousarray(bert_x.astype(ml_dtypes.bfloat16))

    consts, mask01 = _np_consts()
    mask_b = np.ascontiguousarray(mask01.astype(ml_dtypes.bfloat16))

    maps = []
    for k in range(NCORES):
        maps.append(
            {
                "bx": bxb[k * BPC:(k + 1) * BPC].reshape(ROWS, H),
                "eas": EAs[k * BPC:(k + 1) * BPC].reshape(ROWS, 2 * D),
                "consts": consts,
                "mask": mask_b,
            }
        )
    return maps


def _run(bert_x, x, ae, w, trace=False):
    from concourse import bass_utils

    nc = _build()
    maps = _in_maps(bert_x, x, ae, w)
    res = bass_utils.run_bass_kernel_spmd(
        nc, maps, core_ids=list(range(NCORES)), trace=trace
    )
    out = np.concatenate(
        [
            res.results[k]["out"].astype(np.float32).reshape(BPC, L, H)
            for k in range(NCORES)
        ],
        axis=0,
    )
    return out, res


def kernel(bert_x, x, ae, w):
    out, _ = _run(bert_x, x, ae, w, trace=False)
    return out
